# revision 52
# baseline (speedup 1.0000x reference)
# Trainium2 Bass kernel for nn_BertAdapter_SLT_49933289783411
#
# Reference computation:
#   y   = tt_linear(x) + bias          (TT-factorized 768->768 linear)
#   out = x + gelu_exact(y)
#
# Key math: the TT cores with ranks [1,5,5,5,5,5,1] factor the 768x768
# weight as W = A @ B with A:(768,5), B:(5,768).  We precompute A,B on
# host (tiny, exact) and run a rank-5 bottleneck matmul on device.
#
# Sharding: data-parallel over the batch dim (8 batch elements -> 8 cores).
# Each core handles x_c:(512,768), pre-transposed on host to x^T (feature-
# major) so the contraction dim lands on SBUF partitions.  Per core:
#   t3    = A^T @ x^T              (8,512)   PSUM accumulate over f-chunks
#   y^T_j = B_j^T @ t3_pad         (128,512) per 128-feature output chunk j
#   o^T_j = x^T_j + gelu_exact(y^T_j + bias_j)
# The host transposes the gathered o^T back.
#
# All device I/O is bf16 (packed in pairs into f32 DRAM columns): the
# 2e-2 rel-err budget dwarfs bf16 rounding (~2e-3), and halving the HBM
# bytes halves the DMA-bound portion of the schedule.
#
# Schedule notes (from perfetto traces):
#  - The ACT engine's serialized gelu chain (~3.5us) is the longest pipe
#    segment; groups are sized ascending-then-descending (512/1024 heads,
#    256 tails) so it starts as early and drains as late as possible.
#  - Loads split A=consts+c0 / B=c1..c5 / C=half1 across the Sync and
#    Pool DGE queues so mm1's gate (issue+DGE+transfer+sem ~2.9us) is paid
#    on ~1/3 of the bytes.  PE order mm1h0,mm1h1,mm2h0,mm2h1 keeps CAST1
#    off the mm2 h1 critical path.
#  - PSUM is exactly 8 banks: per half a 1-bank (j0,j1) + 2-bank (j2..j5)
#    mm2 tile, two 1-bank t3 tiles; the warmup matmuls write into half 1's
#    2-bank tile before its first real use.
#  - B_pad row 32 carries the bias and meets an all-ones row 32 of t3
#    (gpsimd memset writes 32-aligned partition ranges; B_pad rows 33..63
#    stay zero).  Rank padded 5->8 keeps bf16 A-slices 4B-aligned.

import numpy as np
import ml_dtypes

import concourse.bass as bass
import concourse.bacc as bacc
import concourse.mybir as mybir
import concourse.tile as tile
from concourse.bass_utils import run_bass_kernel_spmd

HID = 768
ROWS = 512          # rows per core (one batch element)
HSIZE = (256, 256)
HOFF = (0, 256)
NCORES = 8
FCH = 6             # 768 / 128 feature chunks
RANKP = 8           # TT rank 5 zero-padded to 8 (bf16 alignment)
F32 = mybir.dt.float32
BF16 = mybir.dt.bfloat16

# dummy PE matmuls to trip the HAM clock un-throttle: sized to keep the
# PE busy until the x-h0 load's completion sem on a median-contention run
# — a PE idle gap >~0.5us there drops the clock to the mid p-state (2x
# slower matmuls) for several microseconds.  The sem arrival jitters by
# ~2us with HBM contention from the other 7 cores, so cover the median:
# ending early costs 2x on every matmul, ending late costs the overshoot.
N_WARMUP = 38

# packed layout of the input tensor, in bf16 columns:
#   [A_pad (128,48)] [B_pad (128,768)] [x h0: c0..c5 x 256] [x h1: ...]
#   [t3 staging (128,512); only row 8 (the all-ones bias row) is real]
A_COLS = FCH * RANKP                               # 48
BM_COLS = HID                                      # 768
CONST_COLS = A_COLS + BM_COLS                      # 816
T3_OFF = CONST_COLS + 2 * FCH * HSIZE[0]           # 3888
XT_COLS = T3_OFF + ROWS                            # 4400 bf16 = 2200 f32
OUT_COLS = FCH * ROWS                              # 3072 bf16 = 1536 f32

# gelu/add/store groups per half: (start_chunk, n_chunks).  Half 0 as
# three pair-ops: its [j4,j5] op bridges the ACT chain across the wait
# for mm2 h1, killing the stall a big [j2..j5] op would expose.  Half 1
# ends 2/3/1 so the final gelu->add->store->sem chain (which the fixed
# ~7us walrus teardown serializes behind) is as short as possible.
GROUPS = (((0, 2), (2, 4)), ((0, 2), (2, 2), (4, 2)))

_CACHE = {}


class _LeanTileContext(tile.TileContext):
    """TileContext with a minimal exit sequence.

    The stock exit emits drain + all-engine barrier + per-sem clears +
    barrier.  The NEFF-level epilogue walrus emits already re-clears the
    whole semaphore space on every execution, so only the drain — which
    makes the kernel end wait for the output DMAs — is kept.
    """

    def _drain_and_barrier(self, tick_clock, wait_clock):
        drain_inst = self.nc.sync.drain()
        wait_clock.add_sem_waits(
            drain_inst.ins, tile.ScopedClock({None: tick_clock.global_clock})
        )
        popped = self.nc._tile_sem_poison_stack.pop()
        assert popped is self._sem_poison


def _xcol(h, c):
    # column (in bf16 units) of x half h, chunk c
    return CONST_COLS + FCH * HOFF[h] + c * HSIZE[h]


def _ocol(h, j):
    return h * FCH * HSIZE[h] + j * HSIZE[h]


def _build_program(act=None):
    if act is None:
        act = mybir.ActivationFunctionType.Gelu
    nc = bacc.Bacc(None, target_bir_lowering=False)
    xt = nc.dram_tensor("xt", [128, XT_COLS // 2], F32, kind="ExternalInput")
    outt = nc.dram_tensor("outt", [128, OUT_COLS // 2], F32, kind="ExternalOutput")

    with _LeanTileContext(nc) as tc:
        with (
            tc.tile_pool(name="const", bufs=1) as cpool,
            tc.tile_pool(name="xs", bufs=1) as xpool,
            tc.tile_pool(name="work", bufs=3) as wpool,
            tc.tile_pool(name="ps_t3", bufs=1, space="PSUM") as tpool,
            tc.tile_pool(name="ps_a", bufs=1, space="PSUM") as apool,
            tc.tile_pool(name="ps_b", bufs=1, space="PSUM") as bpool,
        ):
            x_sb = xpool.tile([128, XT_COLS // 2], F32)
            xb = x_sb[:].bitcast(BF16)                     # (128, XT_COLS)
            a_view = xb[:, 0:A_COLS]                       # (128, 48)
            bm_view = xb[:, A_COLS:CONST_COLS]             # (128, 768)

            o_sb = xpool.tile([128, OUT_COLS // 2], F32)
            ob = o_sb[:].bitcast(BF16)                     # (128, 3072)

            t3_ps = [
                tpool.tile([RANKP, HSIZE[h]], F32, name=f"t3_ps{h}") for h in (0, 1)
            ]
            ps_a = [apool.tile([128, 512], F32, name=f"ps_a{h}") for h in (0, 1)]
            ps_b = [bpool.tile([128, 1024], F32, name=f"ps_b{h}") for h in (0, 1)]

            # --- PE warmup: garbage matmuls so the HAM clock gate opens;
            # the memset on the otherwise-idle DVE gates the chain
            wsb = cpool.tile([128, 128], BF16)
            nc.vector.memset(wsb[:], 0.0)
            warm_mms = []
            for _ in range(N_WARMUP):
                warm_mms.append(
                    nc.tensor.matmul(
                        ps_b[1][:, 0:128], wsb[:], wsb[:], start=True, stop=True
                    )
                )

            # --- loads.  The 16 DMA engines process queued descriptors in
            # arrival order, so two HWDGE queues do NOT overlap transfers
            # under saturation — the mm1-critical x h0 goes as ONE
            # continuous Sync-queue DMA that owns the bus from the start.
            # The tiny weight loads (A_pad full-partition; only rows 0..8
            # of B_pad are real for the K=9 mm2) and the all-ones bias row
            # ride the Pool SWDGE ring — its ~1us descriptor-gen latency is
            # still far ahead of first use.
            a_end = CONST_COLS // 2
            b_end = _xcol(1, 0) // 2
            nc.sync.dma_start(x_sb[:, a_end:b_end], xt[:, a_end:b_end])
            nc.gpsimd.dma_start(x_sb[:, 0 : A_COLS // 2], xt[:, 0 : A_COLS // 2])
            nc.gpsimd.dma_start(
                x_sb[0:9, A_COLS // 2 : CONST_COLS // 2],
                xt[0:9, A_COLS // 2 : CONST_COLS // 2],
            )
            nc.gpsimd.dma_start(
                x_sb[8:9, T3_OFF // 2 :], xt[8:9, T3_OFF // 2 :]
            )
            # h1 on the Activation HWDGE queue in two pieces: halves the PE
            # idle gap if mm1 h1 catches a late-landing transfer (a >1.5us
            # PE stall drops the clock to the mid p-state).  The issue is
            # held behind warmup ~16 so h1 doesn't contend for the HBM bus
            # while the mm1-critical x h0 transfer is in flight.
            c_mid = _xcol(1, 3) // 2
            c1 = nc.scalar.dma_start(x_sb[:, b_end:c_mid], xt[:, b_end:c_mid])
            tile.add_dep_helper(
                c1.ins, warm_mms[15].ins, sync=True,
                reason="delay h1 load past the x h0 transfer",
            )
            nc.scalar.dma_start(
                x_sb[:, c_mid : T3_OFF // 2], xt[:, c_mid : T3_OFF // 2]
            )

            # t3 in bf16 on 9 partitions of the staging area: rows 0..7 the
            # (padded) TT rank written by the CASTs, row 8 all-ones from the
            # host — paired with the bias in B_pad's row 8 it folds the TT
            # bias into mm2, which runs K=9 (ldweights 9 rows, not 128).
            t3v = xb[:, T3_OFF : T3_OFF + ROWS]

            # mm1/cast/mm2 run as two 128-column sub-passes per half: the
            # PSUM->SBUF cast of sub 0 overlaps mm1's sub-1 accumulation,
            # and mm2 does all sub-0 matmuls first, so the first gelu's
            # gate is mm1-end + ~0.55us instead of +0.9us (K=9 makes the
            # doubled ldweights free).
            def mm1_half(h, after=None):
                mms = []
                for sub in (0, 1):
                    for c in range(FCH):
                        x0 = _xcol(h, c) + sub * 128
                        mm = nc.tensor.matmul(
                            t3_ps[h][:, sub * 128 : (sub + 1) * 128],
                            a_view[:, c * RANKP : (c + 1) * RANKP],
                            xb[:, x0 : x0 + 128],
                            start=(c == 0),
                            stop=(c == FCH - 1),
                        )
                        mms.append(mm)
                        if after is not None:
                            # ordering-only edge: the Tile scheduler
                            # otherwise hoists these DMA-gated matmuls ahead
                            # of mm2 h0, delaying half 0's gelu chain when
                            # the h1 load (last on the bus) lands late
                            tile.add_dep_helper(
                                mm.ins, after.ins, sync=False,
                                reason="mm1 h1 after mm2 h0",
                            )
                return mms

            def cast_half(h):
                for sub in (0, 1):
                    t0 = HOFF[h] + sub * 128
                    nc.vector.tensor_copy(
                        t3v[0:RANKP, t0 : t0 + 128],
                        t3_ps[h][:, sub * 128 : (sub + 1) * 128],
                    )

            def mm2_half(h):
                sz, off = HSIZE[h], HOFF[h]
                mms = []
                for sub in (0, 1):
                    for j in range(FCH):
                        ps, q = (ps_a[h], j) if j < 2 else (ps_b[h], j - 2)
                        t0 = off + sub * 128
                        mm = nc.tensor.matmul(
                            ps[:, q * sz + sub * 128 : q * sz + sub * 128 + 128],
                            bm_view[0 : RANKP + 1, j * 128 : (j + 1) * 128],
                            t3v[0 : RANKP + 1, t0 : t0 + 128],
                            start=(sub == 0 and q % 2 == 0),
                            stop=(sub == 1 and q % 2 == 1),
                        )
                        mms.append(mm)
                return mms

            def out_group(h, j0, nj):
                # gelu -> +x -> store for chunks [j0, j0+nj) of half h
                sz = HSIZE[h]
                w = nj * sz
                if j0 < 2:
                    src = ps_a[h][:, j0 * sz : (j0 + nj) * sz]
                else:
                    src = ps_b[h][:, (j0 - 2) * sz : (j0 - 2 + nj) * sz]
                g_sb = wpool.tile([128, 1024], BF16, tag="g_sb", bufs=3)
                nc.scalar.activation(g_sb[:, :w], src, act, scale=1.0)
                oslice = ob[:, _ocol(h, j0) : _ocol(h, j0) + w]
                nc.vector.tensor_add(
                    oslice, g_sb[:, :w], xb[:, _xcol(h, j0) : _xcol(h, j0) + w]
                )
                # only the earliest-slack groups ride the Pool SWDGE queue:
                # its ~1us descriptor-gen latency put a late group's
                # completion sem right at the end-of-kernel drain gate
                dma = nc.gpsimd if out_group.idx in (0, 2) else nc.sync
                out_group.idx += 1
                dma.dma_start(
                    outt[:, _ocol(h, j0) // 2 : (_ocol(h, j0) + w) // 2],
                    o_sb[:, _ocol(h, j0) // 2 : (_ocol(h, j0) + w) // 2],
                )

            out_group.idx = 0

            # PE order mm1h0, mm2h0, mm1h1, mm2h1: half 1's x lands near the
            # end of the (bus-bound) load phase, so mm1h1 must not sit ahead
            # of mm2h0 in the strict PE FIFO.  CAST1 is emitted before half
            # 0's adds so the DVE doesn't delay mm2h1 behind them.
            def fillers(n, after):
                # keep the PE busy across a CAST-wait gap (~0.45us): an idle
                # PE drops to the mid p-state within ~0.5us and takes a few
                # ops to re-ramp.  Ordering-only pinned after `after`.
                for _ in range(n):
                    mm = nc.tensor.matmul(
                        ps_b[1][:, 0:128], wsb[:], wsb[:], start=True, stop=True
                    )
                    tile.add_dep_helper(
                        mm.ins, after.ins, sync=False, reason="p-state filler"
                    )

            mm1_h0 = mm1_half(0)
            cast_half(0)
            fillers(4, mm1_h0[-1])
            mm2_h0 = mm2_half(0)
            mm1_h1 = mm1_half(1, after=mm2_h0[-1])
            cast_half(1)
            fillers(4, mm1_h1[-1])
            for j0, nj in GROUPS[0]:
                out_group(0, j0, nj)
            mm2_half(1)
            for j0, nj in GROUPS[1]:
                out_group(1, j0, nj)

    nc.finalize()
    return nc


def _get_program():
    if "nc" not in _CACHE:
        _CACHE["nc"] = _build_program()
    return _CACHE["nc"]


def _host_prep(hidden_states, bias, cores):
    """Collapse TT cores to rank-5 factors; pack consts + x^T per core."""
    c0, c1, c2, c3, c4, c5 = [c.astype(np.float64) for c in cores]
    A = np.einsum("iv,vjw,wkx->ijkx", c0[0], c1, c2).reshape(HID, 5)
    Bm = np.einsum("xpy,yqz,zr->xpqr", c3, c4, c5[:, :, 0]).reshape(5, HID)

    a_p = np.zeros((128, FCH, RANKP), dtype=ml_dtypes.bfloat16)
    a_p[:, :, :5] = A.reshape(FCH, 128, 5).transpose(1, 0, 2)
    a_p = a_p.reshape(128, A_COLS)
    bm_pad = np.zeros((128, HID), dtype=ml_dtypes.bfloat16)
    bm_pad[:5] = Bm.astype(ml_dtypes.bfloat16)
    # row 8 carries the TT bias; it meets the all-ones row 8 of t3 in mm2
    bm_pad[RANKP] = bias.astype(ml_dtypes.bfloat16)
    t3_stage = np.zeros((128, ROWS), dtype=ml_dtypes.bfloat16)
    t3_stage[RANKP] = 1.0

    const_block = np.concatenate([a_p, bm_pad], axis=1)   # (128, 816) bf16

    xts = []
    for c in range(NCORES):
        xct = hidden_states[c].T.astype(ml_dtypes.bfloat16)   # (768, 512)
        # [p, h*1536 + c*256 + m~] = x^T[c*128+p, h*256+m~]
        xr = (
            xct.reshape(FCH, 128, 2, HSIZE[0])
            .transpose(1, 2, 0, 3)
            .reshape(128, 2 * FCH * HSIZE[0])
        )
        packed = np.concatenate([const_block, xr, t3_stage], axis=1)  # (128, 4400)
        xts.append(np.ascontiguousarray(packed).view(np.float32))
    return xts


def _unpack_out(outt_list):
    """outt[p, h*1536 + j*256 + m~] = out[h*256+m~, j*128+p] -> (8,512,768)."""
    outs = []
    for outt in outt_list:
        ob = np.ascontiguousarray(outt).view(ml_dtypes.bfloat16)
        o = (
            ob.reshape(128, 2, FCH, HSIZE[0])
            .transpose(1, 3, 2, 0)
            .reshape(ROWS, HID)
        )
        outs.append(o.astype(np.float32))
    return np.stack(outs, axis=0)


def run(inputs, trace=False, **spmd_kwargs):
    hidden_states = np.asarray(inputs["hidden_states"], dtype=np.float32)
    bias = np.asarray(inputs["bias"], dtype=np.float32)
    cores = [np.asarray(inputs[f"core{i}"], dtype=np.float32) for i in range(6)]

    xts = _host_prep(hidden_states, bias, cores)
    nc = _get_program()
    in_maps = [{"xt": xts[c]} for c in range(NCORES)]
    res = run_bass_kernel_spmd(
        nc, in_maps, core_ids=list(range(NCORES)), trace=trace, **spmd_kwargs
    )
    out = _unpack_out([res.results[c]["outt"] for c in range(NCORES)])
    if trace:
        return out, res
    return out


def kernel(**inputs):
    return run(inputs)


# revision 54
# speedup vs baseline: 1.0389x; 1.0389x over previous
# Trainium2 Bass kernel for nn_BertAdapter_SLT_49933289783411
#
# Reference computation:
#   y   = tt_linear(x) + bias          (TT-factorized 768->768 linear)
#   out = x + gelu_exact(y)
#
# Key math: the TT cores with ranks [1,5,5,5,5,5,1] factor the 768x768
# weight as W = A @ B with A:(768,5), B:(5,768).  We precompute A,B on
# host (tiny, exact) and run a rank-5 bottleneck matmul on device.
#
# Sharding: data-parallel over the batch dim (8 batch elements -> 8 cores).
# Each core handles x_c:(512,768), pre-transposed on host to x^T (feature-
# major) so the contraction dim lands on SBUF partitions.  Per core:
#   t3    = A^T @ x^T              (8,512)   PSUM accumulate over f-chunks
#   y^T_j = B_j^T @ t3_pad         (128,512) per 128-feature output chunk j
#   o^T_j = x^T_j + gelu_exact(y^T_j + bias_j)
# The host transposes the gathered o^T back.
#
# All device I/O is bf16 (packed in pairs into f32 DRAM columns): the
# 2e-2 rel-err budget dwarfs bf16 rounding (~2e-3), and halving the HBM
# bytes halves the DMA-bound portion of the schedule.
#
# Schedule notes (from perfetto traces):
#  - The ACT engine's serialized gelu chain (~3.5us) is the longest pipe
#    segment; groups are sized ascending-then-descending (512/1024 heads,
#    256 tails) so it starts as early and drains as late as possible.
#  - Loads split A=consts+c0 / B=c1..c5 / C=half1 across the Sync and
#    Pool DGE queues so mm1's gate (issue+DGE+transfer+sem ~2.9us) is paid
#    on ~1/3 of the bytes.  PE order mm1h0,mm1h1,mm2h0,mm2h1 keeps CAST1
#    off the mm2 h1 critical path.
#  - PSUM is exactly 8 banks: per half a 1-bank (j0,j1) + 2-bank (j2..j5)
#    mm2 tile, two 1-bank t3 tiles; the warmup matmuls write into half 1's
#    2-bank tile before its first real use.
#  - B_pad row 32 carries the bias and meets an all-ones row 32 of t3
#    (gpsimd memset writes 32-aligned partition ranges; B_pad rows 33..63
#    stay zero).  Rank padded 5->8 keeps bf16 A-slices 4B-aligned.

import numpy as np
import ml_dtypes

import concourse.bass as bass
import concourse.bacc as bacc
import concourse.mybir as mybir
import concourse.tile as tile
from concourse.bass_utils import run_bass_kernel_spmd

HID = 768
ROWS = 512          # rows per core (one batch element)
HSIZE = (256, 256)
HOFF = (0, 256)
NCORES = 8
FCH = 6             # 768 / 128 feature chunks
RANKP = 8           # TT rank 5 zero-padded to 8 (bf16 alignment)
F32 = mybir.dt.float32
BF16 = mybir.dt.bfloat16

# dummy PE matmuls to trip the HAM clock un-throttle: sized to keep the
# PE busy until the x-h0 load's completion sem on a median-contention run
# — a PE idle gap >~0.5us there drops the clock to the mid p-state (2x
# slower matmuls) for several microseconds.  The sem arrival jitters by
# ~2us with HBM contention from the other 7 cores, so cover the median:
# ending early costs 2x on every matmul, ending late costs the overshoot.
N_WARMUP = 38

# packed layout of the input tensor, in bf16 columns:
#   [A_pad (128,48)] [B_pad (128,768)] [x h0: c0..c5 x 256] [x h1: ...]
#   [t3 staging (128,512); only row 8 (the all-ones bias row) is real]
A_COLS = FCH * RANKP                               # 48
BM_COLS = HID                                      # 768
CONST_COLS = A_COLS + BM_COLS                      # 816
T3_OFF = CONST_COLS + 2 * FCH * HSIZE[0]           # 3888
XT_COLS = T3_OFF + ROWS                            # 4400 bf16 = 2200 f32
OUT_COLS = FCH * ROWS                              # 3072 bf16 = 1536 f32

# gelu/add/store groups per half: (start_chunk, n_chunks).  Half 0 as
# three pair-ops: its [j4,j5] op bridges the ACT chain across the wait
# for mm2 h1, killing the stall a big [j2..j5] op would expose.  Half 1
# ends 2/3/1 so the final gelu->add->store->sem chain (which the fixed
# ~7us walrus teardown serializes behind) is as short as possible.
GROUPS = (((0, 2), (2, 4)), ((0, 2), (2, 2), (4, 2)))

_CACHE = {}


class _LeanTileContext(tile.TileContext):
    """TileContext with a minimal exit sequence.

    The stock exit emits drain + all-engine barrier + per-sem clears +
    barrier.  The NEFF-level epilogue walrus emits already re-clears the
    whole semaphore space on every execution, so only the drain — which
    makes the kernel end wait for the output DMAs — is kept.
    """

    def _drain_and_barrier(self, tick_clock, wait_clock):
        drain_inst = self.nc.sync.drain()
        wait_clock.add_sem_waits(
            drain_inst.ins, tile.ScopedClock({None: tick_clock.global_clock})
        )
        popped = self.nc._tile_sem_poison_stack.pop()
        assert popped is self._sem_poison


def _xcol(h, c):
    # column (in bf16 units) of x half h, chunk c
    return CONST_COLS + FCH * HOFF[h] + c * HSIZE[h]


def _ocol(h, j):
    return h * FCH * HSIZE[h] + j * HSIZE[h]


def _build_program(act=None):
    if act is None:
        act = mybir.ActivationFunctionType.Gelu
    nc = bacc.Bacc(None, target_bir_lowering=False)
    xt = nc.dram_tensor("xt", [128, XT_COLS // 2], F32, kind="ExternalInput")
    outt = nc.dram_tensor("outt", [128, OUT_COLS // 2], F32, kind="ExternalOutput")

    with _LeanTileContext(nc) as tc:
        with (
            tc.tile_pool(name="const", bufs=1) as cpool,
            tc.tile_pool(name="xs", bufs=1) as xpool,
            tc.tile_pool(name="work", bufs=3) as wpool,
            tc.tile_pool(name="ps_t3", bufs=1, space="PSUM") as tpool,
            tc.tile_pool(name="ps_a", bufs=1, space="PSUM") as apool,
            tc.tile_pool(name="ps_b", bufs=1, space="PSUM") as bpool,
        ):
            x_sb = xpool.tile([128, XT_COLS // 2], F32)
            xb = x_sb[:].bitcast(BF16)                     # (128, XT_COLS)
            a_view = xb[:, 0:A_COLS]                       # (128, 48)
            bm_view = xb[:, A_COLS:CONST_COLS]             # (128, 768)

            o_sb = xpool.tile([128, OUT_COLS // 2], F32)
            ob = o_sb[:].bitcast(BF16)                     # (128, 3072)

            t3_ps = [
                tpool.tile([RANKP, HSIZE[h]], F32, name=f"t3_ps{h}") for h in (0, 1)
            ]
            ps_a = [apool.tile([128, 512], F32, name=f"ps_a{h}") for h in (0, 1)]
            ps_b = [bpool.tile([128, 1024], F32, name=f"ps_b{h}") for h in (0, 1)]

            # --- PE warmup: garbage matmuls so the HAM clock gate opens;
            # the memset on the otherwise-idle DVE gates the chain
            wsb = cpool.tile([128, 128], BF16)
            nc.vector.memset(wsb[:], 0.0)
            warm_mms = []
            for _ in range(N_WARMUP):
                warm_mms.append(
                    nc.tensor.matmul(
                        ps_b[1][:, 0:128], wsb[:], wsb[:], start=True, stop=True
                    )
                )

            # --- loads.  The 16 DMA engines process queued descriptors in
            # arrival order, so two HWDGE queues do NOT overlap transfers
            # under saturation — the mm1-critical x h0 goes as ONE
            # continuous Sync-queue DMA that owns the bus from the start.
            # The tiny weight loads (A_pad full-partition; only rows 0..8
            # of B_pad are real for the K=9 mm2) and the all-ones bias row
            # ride the Pool SWDGE ring — its ~1us descriptor-gen latency is
            # still far ahead of first use.
            a_end = CONST_COLS // 2
            b_end = _xcol(1, 0) // 2
            nc.sync.dma_start(x_sb[:, a_end:b_end], xt[:, a_end:b_end])
            nc.gpsimd.dma_start(x_sb[:, 0 : A_COLS // 2], xt[:, 0 : A_COLS // 2])
            nc.gpsimd.dma_start(
                x_sb[0:9, A_COLS // 2 : CONST_COLS // 2],
                xt[0:9, A_COLS // 2 : CONST_COLS // 2],
            )
            nc.gpsimd.dma_start(
                x_sb[8:9, T3_OFF // 2 :], xt[8:9, T3_OFF // 2 :]
            )
            # h1 on the Activation HWDGE queue in two pieces: halves the PE
            # idle gap if mm1 h1 catches a late-landing transfer (a >1.5us
            # PE stall drops the clock to the mid p-state).  The issue is
            # held behind warmup ~16 so h1 doesn't contend for the HBM bus
            # while the mm1-critical x h0 transfer is in flight.
            c_mid = _xcol(1, 3) // 2
            c1 = nc.scalar.dma_start(x_sb[:, b_end:c_mid], xt[:, b_end:c_mid])
            tile.add_dep_helper(
                c1.ins, warm_mms[15].ins, sync=True,
                reason="delay h1 load past the x h0 transfer",
            )
            nc.scalar.dma_start(
                x_sb[:, c_mid : T3_OFF // 2], xt[:, c_mid : T3_OFF // 2]
            )

            # t3 in bf16 on 9 partitions of the staging area: rows 0..7 the
            # (padded) TT rank written by the CASTs, row 8 all-ones from the
            # host — paired with the bias in B_pad's row 8 it folds the TT
            # bias into mm2, which runs K=9 (ldweights 9 rows, not 128).
            t3v = xb[:, T3_OFF : T3_OFF + ROWS]

            def mm1_half(h, after=None):
                mms = []
                for c in range(FCH):
                    mm = nc.tensor.matmul(
                        t3_ps[h][:],
                        a_view[:, c * RANKP : (c + 1) * RANKP],
                        xb[:, _xcol(h, c) : _xcol(h, c) + HSIZE[h]],
                        start=(c == 0),
                        stop=(c == FCH - 1),
                    )
                    mms.append(mm)
                    if after is not None:
                        # ordering-only edge: the Tile scheduler otherwise
                        # hoists these DMA-gated matmuls ahead of mm2 h0,
                        # delaying half 0's gelu chain when the h1 load
                        # (last on the saturated bus) lands late
                        tile.add_dep_helper(
                            mm.ins, after.ins, sync=False,
                            reason="mm1 h1 after mm2 h0",
                        )
                return mms

            def mm2_half(h):
                sz, off = HSIZE[h], HOFF[h]
                mms = []
                for j in range(FCH):
                    ps, q = (ps_a[h], j) if j < 2 else (ps_b[h], j - 2)
                    mm = nc.tensor.matmul(
                        ps[:, q * sz : (q + 1) * sz],
                        bm_view[0 : RANKP + 1, j * 128 : (j + 1) * 128],
                        t3v[0 : RANKP + 1, off : off + sz],
                        start=(q % 2 == 0),
                        stop=(q % 2 == 1),
                    )
                    mms.append(mm)
                return mms

            def out_group(h, j0, nj):
                # gelu -> +x -> store for chunks [j0, j0+nj) of half h
                sz = HSIZE[h]
                w = nj * sz
                if j0 < 2:
                    src = ps_a[h][:, j0 * sz : (j0 + nj) * sz]
                else:
                    src = ps_b[h][:, (j0 - 2) * sz : (j0 - 2 + nj) * sz]
                g_sb = wpool.tile([128, 1024], BF16, tag="g_sb", bufs=3)
                nc.scalar.activation(g_sb[:, :w], src, act, scale=1.0)
                oslice = ob[:, _ocol(h, j0) : _ocol(h, j0) + w]
                nc.vector.tensor_add(
                    oslice, g_sb[:, :w], xb[:, _xcol(h, j0) : _xcol(h, j0) + w]
                )
                # only the earliest-slack groups ride the Pool SWDGE queue:
                # its ~1us descriptor-gen latency put a late group's
                # completion sem right at the end-of-kernel drain gate
                dma = nc.gpsimd if out_group.idx in (0, 2) else nc.sync
                out_group.idx += 1
                dma.dma_start(
                    outt[:, _ocol(h, j0) // 2 : (_ocol(h, j0) + w) // 2],
                    o_sb[:, _ocol(h, j0) // 2 : (_ocol(h, j0) + w) // 2],
                )

            out_group.idx = 0

            # PE order mm1h0, mm2h0, mm1h1, mm2h1: half 1's x lands near the
            # end of the (bus-bound) load phase, so mm1h1 must not sit ahead
            # of mm2h0 in the strict PE FIFO.  CAST1 is emitted before half
            # 0's adds so the DVE doesn't delay mm2h1 behind them.
            def fillers(n, after):
                # keep the PE busy across a CAST-wait gap (~0.45us): an idle
                # PE drops to the mid p-state within ~0.5us and takes a few
                # ops to re-ramp.  Ordering-only pinned after `after`.
                for _ in range(n):
                    mm = nc.tensor.matmul(
                        ps_b[1][:, 0:128], wsb[:], wsb[:], start=True, stop=True
                    )
                    tile.add_dep_helper(
                        mm.ins, after.ins, sync=False, reason="p-state filler"
                    )

            mm1_h0 = mm1_half(0)
            nc.vector.tensor_copy(t3v[0:RANKP, 0 : HSIZE[0]], t3_ps[0][:])
            fillers(4, mm1_h0[-1])
            mm2_h0 = mm2_half(0)
            mm1_h1 = mm1_half(1, after=mm2_h0[0])
            nc.vector.tensor_copy(
                t3v[0:RANKP, HOFF[1] : HOFF[1] + HSIZE[1]], t3_ps[1][:]
            )
            fillers(4, mm1_h1[-1])
            for j0, nj in GROUPS[0]:
                out_group(0, j0, nj)
            mm2_half(1)
            for j0, nj in GROUPS[1]:
                out_group(1, j0, nj)

    nc.finalize()
    return nc


def _get_program():
    if "nc" not in _CACHE:
        _CACHE["nc"] = _build_program()
    return _CACHE["nc"]


def _host_prep(hidden_states, bias, cores):
    """Collapse TT cores to rank-5 factors; pack consts + x^T per core."""
    c0, c1, c2, c3, c4, c5 = [c.astype(np.float64) for c in cores]
    A = np.einsum("iv,vjw,wkx->ijkx", c0[0], c1, c2).reshape(HID, 5)
    Bm = np.einsum("xpy,yqz,zr->xpqr", c3, c4, c5[:, :, 0]).reshape(5, HID)

    a_p = np.zeros((128, FCH, RANKP), dtype=ml_dtypes.bfloat16)
    a_p[:, :, :5] = A.reshape(FCH, 128, 5).transpose(1, 0, 2)
    a_p = a_p.reshape(128, A_COLS)
    bm_pad = np.zeros((128, HID), dtype=ml_dtypes.bfloat16)
    bm_pad[:5] = Bm.astype(ml_dtypes.bfloat16)
    # row 8 carries the TT bias; it meets the all-ones row 8 of t3 in mm2
    bm_pad[RANKP] = bias.astype(ml_dtypes.bfloat16)
    t3_stage = np.zeros((128, ROWS), dtype=ml_dtypes.bfloat16)
    t3_stage[RANKP] = 1.0

    const_block = np.concatenate([a_p, bm_pad], axis=1)   # (128, 816) bf16

    xts = []
    for c in range(NCORES):
        xct = hidden_states[c].T.astype(ml_dtypes.bfloat16)   # (768, 512)
        # [p, h*1536 + c*256 + m~] = x^T[c*128+p, h*256+m~]
        xr = (
            xct.reshape(FCH, 128, 2, HSIZE[0])
            .transpose(1, 2, 0, 3)
            .reshape(128, 2 * FCH * HSIZE[0])
        )
        packed = np.concatenate([const_block, xr, t3_stage], axis=1)  # (128, 4400)
        xts.append(np.ascontiguousarray(packed).view(np.float32))
    return xts


def _unpack_out(outt_list):
    """outt[p, h*1536 + j*256 + m~] = out[h*256+m~, j*128+p] -> (8,512,768)."""
    outs = []
    for outt in outt_list:
        ob = np.ascontiguousarray(outt).view(ml_dtypes.bfloat16)
        o = (
            ob.reshape(128, 2, FCH, HSIZE[0])
            .transpose(1, 3, 2, 0)
            .reshape(ROWS, HID)
        )
        outs.append(o.astype(np.float32))
    return np.stack(outs, axis=0)


def run(inputs, trace=False, **spmd_kwargs):
    hidden_states = np.asarray(inputs["hidden_states"], dtype=np.float32)
    bias = np.asarray(inputs["bias"], dtype=np.float32)
    cores = [np.asarray(inputs[f"core{i}"], dtype=np.float32) for i in range(6)]

    xts = _host_prep(hidden_states, bias, cores)
    nc = _get_program()
    in_maps = [{"xt": xts[c]} for c in range(NCORES)]
    res = run_bass_kernel_spmd(
        nc, in_maps, core_ids=list(range(NCORES)), trace=trace, **spmd_kwargs
    )
    out = _unpack_out([res.results[c]["outt"] for c in range(NCORES)])
    if trace:
        return out, res
    return out


def kernel(**inputs):
    return run(inputs)


# revision 55
# speedup vs baseline: 1.0579x; 1.0183x over previous
# Trainium2 Bass kernel for nn_BertAdapter_SLT_49933289783411
#
# Reference computation:
#   y   = tt_linear(x) + bias          (TT-factorized 768->768 linear)
#   out = x + gelu_exact(y)
#
# Key math: the TT cores with ranks [1,5,5,5,5,5,1] factor the 768x768
# weight as W = A @ B with A:(768,5), B:(5,768).  We precompute A,B on
# host (tiny, exact) and run a rank-5 bottleneck matmul on device.
#
# Sharding: data-parallel over the batch dim (8 batch elements -> 8 cores).
# Each core handles x_c:(512,768), pre-transposed on host to x^T (feature-
# major) so the contraction dim lands on SBUF partitions.  Per core:
#   t3    = A^T @ x^T              (8,512)   PSUM accumulate over f-chunks
#   y^T_j = B_j^T @ t3_pad         (128,512) per 128-feature output chunk j
#   o^T_j = x^T_j + gelu_exact(y^T_j + bias_j)
# The host transposes the gathered o^T back.
#
# All device I/O is bf16 (packed in pairs into f32 DRAM columns): the
# 2e-2 rel-err budget dwarfs bf16 rounding (~2e-3), and halving the HBM
# bytes halves the DMA-bound portion of the schedule.
#
# Schedule notes (from perfetto traces):
#  - The ACT engine's serialized gelu chain (~3.5us) is the longest pipe
#    segment; groups are sized ascending-then-descending (512/1024 heads,
#    256 tails) so it starts as early and drains as late as possible.
#  - Loads split A=consts+c0 / B=c1..c5 / C=half1 across the Sync and
#    Pool DGE queues so mm1's gate (issue+DGE+transfer+sem ~2.9us) is paid
#    on ~1/3 of the bytes.  PE order mm1h0,mm1h1,mm2h0,mm2h1 keeps CAST1
#    off the mm2 h1 critical path.
#  - PSUM is exactly 8 banks: per half a 1-bank (j0,j1) + 2-bank (j2..j5)
#    mm2 tile, two 1-bank t3 tiles; the warmup matmuls write into half 1's
#    2-bank tile before its first real use.
#  - B_pad row 32 carries the bias and meets an all-ones row 32 of t3
#    (gpsimd memset writes 32-aligned partition ranges; B_pad rows 33..63
#    stay zero).  Rank padded 5->8 keeps bf16 A-slices 4B-aligned.

import numpy as np
import ml_dtypes

import concourse.bass as bass
import concourse.bacc as bacc
import concourse.mybir as mybir
import concourse.tile as tile
from concourse.bass_utils import run_bass_kernel_spmd

HID = 768
ROWS = 512          # rows per core (one batch element)
HSIZE = (256, 256)
HOFF = (0, 256)
NCORES = 8
FCH = 6             # 768 / 128 feature chunks
RANKP = 8           # TT rank 5 zero-padded to 8 (bf16 alignment)
F32 = mybir.dt.float32
BF16 = mybir.dt.bfloat16

# dummy PE matmuls to trip the HAM clock un-throttle: sized to keep the
# PE busy until the x-h0 load's completion sem on a median-contention run
# — a PE idle gap >~0.5us there drops the clock to the mid p-state (2x
# slower matmuls) for several microseconds.  The sem arrival jitters by
# ~2us with HBM contention from the other 7 cores, so cover the median:
# ending early costs 2x on every matmul, ending late costs the overshoot.
N_WARMUP = 38

# packed layout of the input tensor, in bf16 columns:
#   [A_pad (128,48)] [B_pad (128,768)] [x h0: c0..c5 x 256] [x h1: ...]
#   [t3 staging (128,512); only row 8 (the all-ones bias row) is real]
A_COLS = FCH * RANKP                               # 48
BM_COLS = HID                                      # 768
CONST_COLS = A_COLS + BM_COLS                      # 816
T3_OFF = CONST_COLS + 2 * FCH * HSIZE[0]           # 3888
XT_COLS = T3_OFF + ROWS                            # 4400 bf16 = 2200 f32
OUT_COLS = FCH * ROWS                              # 3072 bf16 = 1536 f32

# gelu/add/store groups per half: (start_chunk, n_chunks).  Half 0 as
# three pair-ops: its [j4,j5] op bridges the ACT chain across the wait
# for mm2 h1, killing the stall a big [j2..j5] op would expose.  Half 1
# ends 2/3/1 so the final gelu->add->store->sem chain (which the fixed
# ~7us walrus teardown serializes behind) is as short as possible.
GROUPS = (((0, 2), (2, 4)), ((0, 2), (2, 2), (4, 2)))

_CACHE = {}


class _LeanTileContext(tile.TileContext):
    """TileContext with a minimal exit sequence.

    The stock exit emits drain + all-engine barrier + per-sem clears +
    barrier.  The NEFF-level epilogue walrus emits already re-clears the
    whole semaphore space on every execution, so only the drain — which
    makes the kernel end wait for the output DMAs — is kept.
    """

    def _drain_and_barrier(self, tick_clock, wait_clock):
        # No sem waits on the drain: the ~7us walrus teardown that follows
        # takes far longer than the in-flight output stores need to land
        # (~4.5us margin measured), and PJRT reads outputs only after the
        # full NEFF completes — so the teardown overlaps the store tail
        # instead of serializing behind the last completion sem.
        self.nc.sync.drain()
        popped = self.nc._tile_sem_poison_stack.pop()
        assert popped is self._sem_poison


def _xcol(h, c):
    # column (in bf16 units) of x half h, chunk c
    return CONST_COLS + FCH * HOFF[h] + c * HSIZE[h]


def _ocol(h, j):
    return h * FCH * HSIZE[h] + j * HSIZE[h]


def _build_program(act=None):
    if act is None:
        act = mybir.ActivationFunctionType.Gelu
    nc = bacc.Bacc(None, target_bir_lowering=False)
    xt = nc.dram_tensor("xt", [128, XT_COLS // 2], F32, kind="ExternalInput")
    outt = nc.dram_tensor("outt", [128, OUT_COLS // 2], F32, kind="ExternalOutput")

    with _LeanTileContext(nc) as tc:
        with (
            tc.tile_pool(name="const", bufs=1) as cpool,
            tc.tile_pool(name="xs", bufs=1) as xpool,
            tc.tile_pool(name="work", bufs=3) as wpool,
            tc.tile_pool(name="ps_t3", bufs=1, space="PSUM") as tpool,
            tc.tile_pool(name="ps_a", bufs=1, space="PSUM") as apool,
            tc.tile_pool(name="ps_b", bufs=1, space="PSUM") as bpool,
        ):
            x_sb = xpool.tile([128, XT_COLS // 2], F32)
            xb = x_sb[:].bitcast(BF16)                     # (128, XT_COLS)
            a_view = xb[:, 0:A_COLS]                       # (128, 48)
            bm_view = xb[:, A_COLS:CONST_COLS]             # (128, 768)

            o_sb = xpool.tile([128, OUT_COLS // 2], F32)
            ob = o_sb[:].bitcast(BF16)                     # (128, 3072)

            t3_ps = [
                tpool.tile([RANKP, HSIZE[h]], F32, name=f"t3_ps{h}") for h in (0, 1)
            ]
            ps_a = [apool.tile([128, 512], F32, name=f"ps_a{h}") for h in (0, 1)]
            ps_b = [bpool.tile([128, 1024], F32, name=f"ps_b{h}") for h in (0, 1)]

            # --- PE warmup: garbage matmuls so the HAM clock gate opens;
            # the memset on the otherwise-idle DVE gates the chain
            wsb = cpool.tile([128, 128], BF16)
            nc.vector.memset(wsb[:], 0.0)
            warm_mms = []
            for _ in range(N_WARMUP):
                warm_mms.append(
                    nc.tensor.matmul(
                        ps_b[1][:, 0:128], wsb[:], wsb[:], start=True, stop=True
                    )
                )

            # --- loads.  The 16 DMA engines process queued descriptors in
            # arrival order, so two HWDGE queues do NOT overlap transfers
            # under saturation — the mm1-critical x h0 goes as ONE
            # continuous Sync-queue DMA that owns the bus from the start.
            # The tiny weight loads (A_pad full-partition; only rows 0..8
            # of B_pad are real for the K=9 mm2) and the all-ones bias row
            # ride the Pool SWDGE ring — its ~1us descriptor-gen latency is
            # still far ahead of first use.
            a_end = CONST_COLS // 2
            b_end = _xcol(1, 0) // 2
            nc.sync.dma_start(x_sb[:, a_end:b_end], xt[:, a_end:b_end])
            nc.gpsimd.dma_start(x_sb[:, 0 : A_COLS // 2], xt[:, 0 : A_COLS // 2])
            nc.gpsimd.dma_start(
                x_sb[0:9, A_COLS // 2 : CONST_COLS // 2],
                xt[0:9, A_COLS // 2 : CONST_COLS // 2],
            )
            nc.gpsimd.dma_start(
                x_sb[8:9, T3_OFF // 2 :], xt[8:9, T3_OFF // 2 :]
            )
            # h1 on the Activation HWDGE queue in two pieces: halves the PE
            # idle gap if mm1 h1 catches a late-landing transfer (a >1.5us
            # PE stall drops the clock to the mid p-state).  The issue is
            # held behind warmup ~16 so h1 doesn't contend for the HBM bus
            # while the mm1-critical x h0 transfer is in flight.
            c_mid = _xcol(1, 3) // 2
            c1 = nc.scalar.dma_start(x_sb[:, b_end:c_mid], xt[:, b_end:c_mid])
            tile.add_dep_helper(
                c1.ins, warm_mms[15].ins, sync=True,
                reason="delay h1 load past the x h0 transfer",
            )
            nc.scalar.dma_start(
                x_sb[:, c_mid : T3_OFF // 2], xt[:, c_mid : T3_OFF // 2]
            )

            # t3 in bf16 on 9 partitions of the staging area: rows 0..7 the
            # (padded) TT rank written by the CASTs, row 8 all-ones from the
            # host — paired with the bias in B_pad's row 8 it folds the TT
            # bias into mm2, which runs K=9 (ldweights 9 rows, not 128).
            t3v = xb[:, T3_OFF : T3_OFF + ROWS]

            def mm1_half(h, after=None):
                mms = []
                for c in range(FCH):
                    mm = nc.tensor.matmul(
                        t3_ps[h][:],
                        a_view[:, c * RANKP : (c + 1) * RANKP],
                        xb[:, _xcol(h, c) : _xcol(h, c) + HSIZE[h]],
                        start=(c == 0),
                        stop=(c == FCH - 1),
                    )
                    mms.append(mm)
                    if after is not None:
                        # ordering-only edge: the Tile scheduler otherwise
                        # hoists these DMA-gated matmuls ahead of mm2 h0,
                        # delaying half 0's gelu chain when the h1 load
                        # (last on the saturated bus) lands late
                        tile.add_dep_helper(
                            mm.ins, after.ins, sync=False,
                            reason="mm1 h1 after mm2 h0",
                        )
                return mms

            def mm2_half(h):
                sz, off = HSIZE[h], HOFF[h]
                mms = []
                for j in range(FCH):
                    ps, q = (ps_a[h], j) if j < 2 else (ps_b[h], j - 2)
                    mm = nc.tensor.matmul(
                        ps[:, q * sz : (q + 1) * sz],
                        bm_view[0 : RANKP + 1, j * 128 : (j + 1) * 128],
                        t3v[0 : RANKP + 1, off : off + sz],
                        start=(q % 2 == 0),
                        stop=(q % 2 == 1),
                    )
                    mms.append(mm)
                return mms

            def out_group(h, j0, nj):
                # gelu -> +x -> store for chunks [j0, j0+nj) of half h
                sz = HSIZE[h]
                w = nj * sz
                if j0 < 2:
                    src = ps_a[h][:, j0 * sz : (j0 + nj) * sz]
                else:
                    src = ps_b[h][:, (j0 - 2) * sz : (j0 - 2 + nj) * sz]
                g_sb = wpool.tile([128, 1024], BF16, tag="g_sb", bufs=3)
                nc.scalar.activation(g_sb[:, :w], src, act, scale=1.0)
                oslice = ob[:, _ocol(h, j0) : _ocol(h, j0) + w]
                nc.vector.tensor_add(
                    oslice, g_sb[:, :w], xb[:, _xcol(h, j0) : _xcol(h, j0) + w]
                )
                # only the earliest-slack groups ride the Pool SWDGE queue:
                # its ~1us descriptor-gen latency put a late group's
                # completion sem right at the end-of-kernel drain gate
                dma = nc.gpsimd if out_group.idx in (0, 2) else nc.sync
                out_group.idx += 1
                dma.dma_start(
                    outt[:, _ocol(h, j0) // 2 : (_ocol(h, j0) + w) // 2],
                    o_sb[:, _ocol(h, j0) // 2 : (_ocol(h, j0) + w) // 2],
                )

            out_group.idx = 0

            # PE order mm1h0, mm2h0, mm1h1, mm2h1: half 1's x lands near the
            # end of the (bus-bound) load phase, so mm1h1 must not sit ahead
            # of mm2h0 in the strict PE FIFO.  CAST1 is emitted before half
            # 0's adds so the DVE doesn't delay mm2h1 behind them.
            def fillers(n, after):
                # keep the PE busy across a CAST-wait gap (~0.45us): an idle
                # PE drops to the mid p-state within ~0.5us and takes a few
                # ops to re-ramp.  Ordering-only pinned after `after`.
                for _ in range(n):
                    mm = nc.tensor.matmul(
                        ps_b[1][:, 0:128], wsb[:], wsb[:], start=True, stop=True
                    )
                    tile.add_dep_helper(
                        mm.ins, after.ins, sync=False, reason="p-state filler"
                    )

            mm1_h0 = mm1_half(0)
            nc.vector.tensor_copy(t3v[0:RANKP, 0 : HSIZE[0]], t3_ps[0][:])
            fillers(4, mm1_h0[-1])
            mm2_h0 = mm2_half(0)
            mm1_h1 = mm1_half(1, after=mm2_h0[0])
            nc.vector.tensor_copy(
                t3v[0:RANKP, HOFF[1] : HOFF[1] + HSIZE[1]], t3_ps[1][:]
            )
            fillers(4, mm1_h1[-1])
            for j0, nj in GROUPS[0]:
                out_group(0, j0, nj)
            mm2_half(1)
            for j0, nj in GROUPS[1]:
                out_group(1, j0, nj)

    nc.finalize()
    return nc


def _get_program():
    if "nc" not in _CACHE:
        _CACHE["nc"] = _build_program()
    return _CACHE["nc"]


def _host_prep(hidden_states, bias, cores):
    """Collapse TT cores to rank-5 factors; pack consts + x^T per core."""
    c0, c1, c2, c3, c4, c5 = [c.astype(np.float64) for c in cores]
    A = np.einsum("iv,vjw,wkx->ijkx", c0[0], c1, c2).reshape(HID, 5)
    Bm = np.einsum("xpy,yqz,zr->xpqr", c3, c4, c5[:, :, 0]).reshape(5, HID)

    a_p = np.zeros((128, FCH, RANKP), dtype=ml_dtypes.bfloat16)
    a_p[:, :, :5] = A.reshape(FCH, 128, 5).transpose(1, 0, 2)
    a_p = a_p.reshape(128, A_COLS)
    bm_pad = np.zeros((128, HID), dtype=ml_dtypes.bfloat16)
    bm_pad[:5] = Bm.astype(ml_dtypes.bfloat16)
    # row 8 carries the TT bias; it meets the all-ones row 8 of t3 in mm2
    bm_pad[RANKP] = bias.astype(ml_dtypes.bfloat16)
    t3_stage = np.zeros((128, ROWS), dtype=ml_dtypes.bfloat16)
    t3_stage[RANKP] = 1.0

    const_block = np.concatenate([a_p, bm_pad], axis=1)   # (128, 816) bf16

    xts = []
    for c in range(NCORES):
        xct = hidden_states[c].T.astype(ml_dtypes.bfloat16)   # (768, 512)
        # [p, h*1536 + c*256 + m~] = x^T[c*128+p, h*256+m~]
        xr = (
            xct.reshape(FCH, 128, 2, HSIZE[0])
            .transpose(1, 2, 0, 3)
            .reshape(128, 2 * FCH * HSIZE[0])
        )
        packed = np.concatenate([const_block, xr, t3_stage], axis=1)  # (128, 4400)
        xts.append(np.ascontiguousarray(packed).view(np.float32))
    return xts


def _unpack_out(outt_list):
    """outt[p, h*1536 + j*256 + m~] = out[h*256+m~, j*128+p] -> (8,512,768)."""
    outs = []
    for outt in outt_list:
        ob = np.ascontiguousarray(outt).view(ml_dtypes.bfloat16)
        o = (
            ob.reshape(128, 2, FCH, HSIZE[0])
            .transpose(1, 3, 2, 0)
            .reshape(ROWS, HID)
        )
        outs.append(o.astype(np.float32))
    return np.stack(outs, axis=0)


def run(inputs, trace=False, **spmd_kwargs):
    hidden_states = np.asarray(inputs["hidden_states"], dtype=np.float32)
    bias = np.asarray(inputs["bias"], dtype=np.float32)
    cores = [np.asarray(inputs[f"core{i}"], dtype=np.float32) for i in range(6)]

    xts = _host_prep(hidden_states, bias, cores)
    nc = _get_program()
    in_maps = [{"xt": xts[c]} for c in range(NCORES)]
    res = run_bass_kernel_spmd(
        nc, in_maps, core_ids=list(range(NCORES)), trace=trace, **spmd_kwargs
    )
    out = _unpack_out([res.results[c]["outt"] for c in range(NCORES)])
    if trace:
        return out, res
    return out


def kernel(**inputs):
    return run(inputs)
